# revision 2
# baseline (speedup 1.0000x reference)
"""Cdist-mean kernel for Trainium2 (8 NeuronCores, SPMD row-sharded).

Computes mean(cdist(x.reshape(T,-1), y.reshape(T,-1))) for T=8192, D=512.

Strategy per core c (of 8): rows x[c*1024:(c+1)*1024] vs all of y.
  sq[i,j] = x2[i] + y2[j] - 2*x.y  via bf16 matmul with K on partitions:
    - 4 matmuls (K=128 chunks) accumulate x.y into PSUM
    - 1 augmented K=1 matmul adds -y2[j]/2 (rhs row precomputed on device)
    - ACT: sqrt(-2*psum + x2[i])  (per-partition bias), with accum_out
      doing the free-dim sum reduction in the same instruction.
  Per-core result: [128, 128] partial sums; host sums and divides by T^2.
"""

import sys

import numpy as np

if "/opt/trn_rl_repo" not in sys.path:
    sys.path.insert(0, "/opt/trn_rl_repo")

import ml_dtypes

T = 8192
D = 512  # flattened feature dim (256*2)
NCORES = 8
M = T // NCORES  # 1024 rows of x per core
P = 128
KC = D // P  # 4 K-chunks
MT = M // P  # 8 m-tiles per core
SEG = 512  # n-segment (matmul free dim)
NSEG = T // SEG  # 16

_CACHE = {}


def _build():
    import concourse.bass as bass
    import concourse.tile as tile
    from concourse import bacc, mybir

    nc = bacc.Bacc(
        "TRN2",
        target_bir_lowering=False,
        debug=False,
        enable_asserts=False,
        num_devices=NCORES,
    )

    xs = nc.dram_tensor("xs", [M, D], mybir.dt.bfloat16, kind="ExternalInput").ap()
    yb = nc.dram_tensor("yb", [T, D], mybir.dt.bfloat16, kind="ExternalInput").ap()
    out = nc.dram_tensor(
        "out", [P, MT * NSEG], mybir.dt.float32, kind="ExternalOutput"
    ).ap()

    with tile.TileContext(nc) as tc:
        with (
            tc.tile_pool(name="persist", bufs=1) as persist,
            tc.tile_pool(name="work", bufs=3) as work,
            tc.tile_pool(name="psum", bufs=4, space="PSUM") as pp,
            tc.tile_pool(name="psum_y2", bufs=2, space="PSUM") as pp_y2,
        ):
            f32 = mybir.dt.float32
            bf16 = mybir.dt.bfloat16

            # ---- persistent tiles ----
            yt = [persist.tile([P, T], bf16, tag=f"yt{kc}", name=f"yt{kc}") for kc in range(KC)]
            xt = [persist.tile([P, M], bf16, tag=f"xt{kc}", name=f"xt{kc}") for kc in range(KC)]
            x_nat = persist.tile([P, MT, D], bf16, tag="x_nat")
            x2_sb = persist.tile([P, MT], f32, tag="x2_sb")
            aug = persist.tile([1, T], bf16, tag="aug")
            acc_cols = persist.tile([P, MT * NSEG], f32, tag="acc_cols")
            ones_col = persist.tile([P, 1], bf16, tag="ones_col")
            ones_row = persist.tile([1, P], bf16, tag="ones_row")

            nc.vector.memset(ones_col[:], 1.0)
            nc.vector.memset(ones_row[:], 1.0)

            # ---- x-side prep ----
            # transposed x chunks: xt[kc][k, m] = x[m, kc*128+k]
            for kc in range(KC):
                nc.sync.dma_start_transpose(
                    xt[kc][:], xs[:, kc * P : (kc + 1) * P]
                )
            # natural x for x2: [p, t, k] = x[t*128+p, k]
            nc.sync.dma_start(
                x_nat[:], xs.rearrange("(t p) k -> p t k", p=P)
            )
            xsq = persist.tile([P, MT, D], f32, tag="xsq")
            nc.vector.tensor_tensor(
                xsq[:], x_nat[:], x_nat[:], mybir.AluOpType.mult
            )
            nc.vector.tensor_reduce(
                x2_sb[:], xsq[:], axis=mybir.AxisListType.X, op=mybir.AluOpType.add
            )

            # ---- y transposes, segment-major so early segments land first ----
            QCH = 4  # column chunks per kc
            QW = T // QCH  # 2048
            for q in range(QCH):
                for kc in range(KC):
                    nc.sync.dma_start_transpose(
                        yt[kc][:, q * QW : (q + 1) * QW],
                        yb[q * QW : (q + 1) * QW, kc * P : (kc + 1) * P],
                    )

            # ---- y2 prep per segment: aug[0, j] = -y2[j]/2 (bf16) ----
            for s in range(NSEG):
                ps_y2 = pp_y2.tile([1, SEG], f32, tag="ps_y2", name="ps_y2")
                for kc in range(KC):
                    ysq = work.tile([P, SEG], bf16, tag="ysq", name="ysq")
                    seg = yt[kc][:, s * SEG : (s + 1) * SEG]
                    nc.vector.tensor_tensor(
                        ysq[:], seg, seg, mybir.AluOpType.mult
                    )
                    nc.tensor.matmul(
                        ps_y2[:],
                        ones_col[:],
                        ysq[:],
                        start=(kc == 0),
                        stop=(kc == KC - 1),
                    )
                nc.scalar.activation(
                    aug[0:1, s * SEG : (s + 1) * SEG],
                    ps_y2[:],
                    mybir.ActivationFunctionType.Copy,
                    scale=-0.5,
                )

            # ---- main loop ----
            for mi in range(MT):
                for ni in range(NSEG):
                    psum = pp.tile([P, SEG], f32, tag="psum", name="psum")
                    for kc in range(KC):
                        nc.tensor.matmul(
                            psum[:],
                            xt[kc][:, mi * P : (mi + 1) * P],
                            yt[kc][:, ni * SEG : (ni + 1) * SEG],
                            start=(kc == 0),
                            stop=False,
                        )
                    nc.tensor.matmul(
                        psum[:],
                        ones_row[:],
                        aug[0:1, ni * SEG : (ni + 1) * SEG],
                        start=False,
                        stop=True,
                    )
                    col = mi * NSEG + ni
                    nc.scalar.activation(
                        psum[:],
                        psum[:],
                        mybir.ActivationFunctionType.Sqrt,
                        bias=x2_sb[:, mi : mi + 1],
                        scale=-2.0,
                        accum_out=acc_cols[:, col : col + 1],
                    )

            nc.sync.dma_start(out[:], acc_cols[:])

    nc.compile()
    return nc


def _get_nc():
    if "nc" not in _CACHE:
        _CACHE["nc"] = _build()
    return _CACHE["nc"]


def _run(x, y, trace=False, **kw):
    from concourse.bass_utils import run_bass_kernel_spmd

    xf = np.ascontiguousarray(np.asarray(x, dtype=np.float32).reshape(T, D))
    yf = np.ascontiguousarray(np.asarray(y, dtype=np.float32).reshape(T, D))
    xb = xf.astype(ml_dtypes.bfloat16)
    ybv = yf.astype(ml_dtypes.bfloat16)

    nc = _get_nc()
    in_maps = [
        {"xs": np.ascontiguousarray(xb[c * M : (c + 1) * M]), "yb": ybv}
        for c in range(NCORES)
    ]
    res = run_bass_kernel_spmd(
        nc, in_maps, core_ids=list(range(NCORES)), trace=trace, **kw
    )
    total = sum(float(r["out"].astype(np.float64).sum()) for r in res.results)
    val = np.float32(total / (float(T) * float(T)))
    return np.array(val, dtype=np.float32), res


def kernel(x, y):
    out, _ = _run(x, y)
    return out
